# revision 10
# baseline (speedup 1.0000x reference)
"""Trainium2 Bass kernel for nn_LitToClauseLayer (gather + segment_sum + LSTM cell).

Reference computation:
    msg   = segment_sum(x_l[edge_lit], edge_clause, num_segments=N_CLAUSE)   # [NC, D]
    gates = msg @ W_ih.T + b_ih + h0 @ W_hh.T + b_hh                         # [NC, 4D]
    i, f, g, o = split(gates); i,f,o = sigmoid; g = tanh
    c_new = f*c0 + i*g ; h_new = o*tanh(c_new)
    returns (h_new, c_new)

Distribution (8 cores, SPMD): clauses + clause-sorted edges sharded across
cores; x_l (fp16) and LSTM weights replicated.  On-device layout is
feature-major ([D=128 partitions, clause]).

v5 structure (vs v4 baseline):
  - Gather calls are packed to up to MAX_CALL_IDX indices each (v4 used
    <=1024 because single_packet=True caps at 64 descs/engine) so the
    ~1us fixed SWDGE ucode overhead per call amortizes.
  - Work is grouped by G=16 clause chunks (2048 clauses): one gather tile,
    one one-hot is_equal, one h0/c0 load, batched pointwise + tanh ops and
    one output store per group; gates stream 512 cols per matmul.
"""

import numpy as np

N_LIT, N_CLAUSE, N_EDGES, D = 100000, 400000, 1200000, 128
N_CORES = 8
CPC = N_CLAUSE // N_CORES  # clauses per core (50000)
P = 128
BANK_ROWS = 25000
G = 16                     # chunks per group
SCC = 4                    # chunks per superchunk (PSUM msg tile)
MAX_CALL_IDX = 512         # pack gather calls up to this many indices
                           # (single_packet caps at 16KB/engine = 1024x256B;
                           # ring is 16KB/queue so ~512 keeps 2 calls in flight)
SINGLE_PACKET = True
NEG_PAD = False            # pad slots use idx=-1 (probe: ucode skip?)
DMA_SCRATCH = 16384        # descriptor ring carveout bytes/partition

_cache = {}

TRACE = False
LAST_RESULT = None


def _compute_structure(edge_lit, edge_clause, n_lit=N_LIT, cpc=CPC,
                       bank_rows=BANK_ROWS, n_cores=N_CORES):
    """Shared (SPMD) program structure + per-core edge placement."""
    n_banks = -(-n_lit // bank_rows)
    n_chunks = -(-cpc // P)
    n_chunks = -(-n_chunks // G) * G
    n_groups = n_chunks // G

    counts = np.zeros((n_cores, n_chunks, n_banks), dtype=np.int64)
    per_core = []
    for k in range(n_cores):
        marks = k * cpc + P * np.arange(n_chunks + 1)
        marks = np.minimum(marks, (k + 1) * cpc)
        bounds = np.searchsorted(edge_clause, marks, side="left")
        cells = {}
        for j in range(n_chunks):
            e0, e1 = bounds[j], bounds[j + 1]
            if e1 > e0:
                lits = edge_lit[e0:e1]
                locs = (edge_clause[e0:e1] - (k * cpc + j * P)).astype(np.int16)
                banks = lits // bank_rows
                order = np.argsort(banks, kind="stable")
                lits, locs, banks = lits[order], locs[order], banks[order]
                for b in range(n_banks):
                    m = banks == b
                    if m.any():
                        cells[(j, b)] = (lits[m] - b * bank_rows, locs[m])
                        counts[k, j, b] = int(m.sum())
        per_core.append(cells)

    maxc = counts.max(axis=0)                       # [n_chunks, n_banks]
    budgets = ((maxc + P - 1) // P) * P             # slots per cell
    for j in range(n_chunks):                       # every chunk >= 1 tile
        if budgets[j].sum() == 0:
            budgets[j, 0] = P

    # Pack gather calls: per (group, bank), consecutive chunk-cells packed
    # into calls of <= MAX_CALL_IDX indices (cell budgets are 128-multiples).
    # calls: list of (group, bank, chunk_lo, chunk_hi, nidx)
    calls = []
    for g in range(n_groups):
        for b in range(n_banks):
            j = g * G
            while j < (g + 1) * G:
                nidx = 0
                j0 = j
                while j < (g + 1) * G and nidx + budgets[j, b] <= MAX_CALL_IDX:
                    nidx += int(budgets[j, b])
                    j += 1
                assert j > j0, f"cell budget {budgets[j, b]} > MAX_CALL_IDX"
                if nidx > 0:
                    calls.append((g, b, j0, j, nidx))

    structure = {
        "n_lit": n_lit, "n_banks": n_banks, "bank_rows": bank_rows,
        "n_chunks": n_chunks, "n_groups": n_groups,
        "budgets": budgets, "calls": calls,
    }
    return structure, per_core


def _group_geometry(st):
    """Per-group tile lists and index-column layout (compile-time, SPMD).

    Tile order within a group: bank-major, then chunk, matching the call
    packing so each call writes a contiguous tile range of the group tile.
    Returns per-group dicts.
    """
    budgets = st["budgets"]
    n_banks, n_groups = st["n_banks"], st["n_groups"]
    geoms = []
    icol = 0        # global eidx column (16 idx per column)
    tcol = 0        # global eloc tile index
    for g in range(n_groups):
        tiles = []          # (bank, chunk_global) per 128-slot tile
        gcalls = []         # (bank, tile_lo, tile_hi, icol_lo, nidx, chunk_lo, chunk_hi)
        icol0, tcol0 = icol, tcol
        for (cg, b, j0, j1, nidx) in st["calls"]:
            if cg != g:
                continue
            tile_lo = len(tiles)
            for j in range(j0, j1):
                for _ in range(int(budgets[j, b]) // P):
                    tiles.append((b, j))
            gcalls.append((b, tile_lo, len(tiles), icol - icol0, nidx, j0, j1))
            icol += nidx // 16
        tcol += len(tiles)
        geoms.append({
            "tiles": tiles, "calls": gcalls,
            "icol0": icol0, "icols": icol - icol0,
            "tcol0": tcol0, "ntile": len(tiles),
        })
    return geoms, icol, tcol


def _build_program(st):
    import concourse.bacc as bacc
    import concourse.bass as bass
    import concourse.mybir as mybir
    import concourse.tile as tile

    dt = mybir.dt
    n_chunks, n_groups = st["n_chunks"], st["n_groups"]
    n_banks, bank_rows = st["n_banks"], st["bank_rows"]
    budgets = st["budgets"]
    n_lit = st["n_lit"]
    ncols = n_chunks * P
    gcols = G * P
    sc_per_g = G // SCC

    geoms, total_icols, total_tiles = _group_geometry(st)
    max_icols = max(gm["icols"] for gm in geoms)
    max_ntile = max(gm["ntile"] for gm in geoms)

    nc = bacc.Bacc(None, target_bir_lowering=False, num_swdge_queues=4,
                   dynamic_dma_scratch_size=DMA_SCRATCH)

    xlb = nc.dram_tensor("xlb", [n_lit, D], dt.float16, kind="ExternalInput")
    h0t = nc.dram_tensor("h0t", [P, ncols], dt.float16, kind="ExternalInput")
    c0t = nc.dram_tensor("c0t", [P, ncols], dt.float16, kind="ExternalInput")
    eidx = nc.dram_tensor("eidx", [P, total_icols], dt.int16, kind="ExternalInput")
    eloc = nc.dram_tensor("eloc", [P, total_tiles], dt.float16, kind="ExternalInput")
    wih = nc.dram_tensor("wih", [P, 4 * D], dt.float16, kind="ExternalInput")
    whh = nc.dram_tensor("whh", [P, 4 * D], dt.float16, kind="ExternalInput")
    btab = nc.dram_tensor("btab", [P, 4], dt.float32, kind="ExternalInput")
    iota = nc.dram_tensor("iota", [P, P], dt.float16, kind="ExternalInput")
    ht = nc.dram_tensor("ht", [P, ncols], dt.float16, kind="ExternalOutput")
    ct = nc.dram_tensor("ct", [P, ncols], dt.float16, kind="ExternalOutput")

    f32, fp16, i16 = dt.float32, dt.float16, dt.int16
    Sig = mybir.ActivationFunctionType.Sigmoid
    Tnh = mybir.ActivationFunctionType.Tanh

    with tile.TileContext(nc) as tc:
        with (
            tc.tile_pool(name="const", bufs=1) as cpool,
            tc.tile_pool(name="tabs", bufs=2) as tabs,
            tc.tile_pool(name="gat", bufs=2) as gat,
            tc.tile_pool(name="oh", bufs=2) as ohp,
            tc.tile_pool(name="hc", bufs=2) as hcp,
            tc.tile_pool(name="msg", bufs=2) as msgp,
            tc.tile_pool(name="pln", bufs=2) as plnp,
            tc.tile_pool(name="tmp", bufs=2) as tmpp,
            tc.tile_pool(name="outs", bufs=2) as outp,
            tc.tile_pool(name="pm", bufs=2, space="PSUM") as pmsum,
            tc.tile_pool(name="pg", bufs=1, space="PSUM") as pgate,
        ):
            wih_sb = cpool.tile([P, 4 * D], fp16, tag="wih")
            whh_sb = cpool.tile([P, 4 * D], fp16, tag="whh")
            b_sb = cpool.tile([P, 4], f32, tag="btab")
            iota_sb = cpool.tile([P, P], fp16, tag="iota")
            nc.sync.dma_start(out=wih_sb[:], in_=wih[:, :])
            nc.sync.dma_start(out=whh_sb[:], in_=whh[:, :])
            nc.sync.dma_start(out=b_sb[:], in_=btab[:, :])
            nc.sync.dma_start(out=iota_sb[:], in_=iota[:, :])

            for g in range(n_groups):
                gm = geoms[g]
                n_t = gm["ntile"]
                icols = gm["icols"]

                idx_t = tabs.tile([P, max(max_icols, 16)], i16, tag="idx")
                nc.sync.dma_start(out=idx_t[:, :icols],
                                  in_=eidx[:, gm["icol0"]:gm["icol0"] + icols])
                elo_t = tabs.tile([P, max_ntile], fp16, tag="elo")
                nc.sync.dma_start(out=elo_t[:, :n_t],
                                  in_=eloc[:, gm["tcol0"]:gm["tcol0"] + n_t])

                g_t = gat.tile([P, max_ntile, D], fp16, tag="gt")
                for (b, t_lo, t_hi, ic_lo, nidx, j0, j1) in gm["calls"]:
                    lo = b * bank_rows
                    hi = min(lo + bank_rows, n_lit)
                    nc.gpsimd.dma_gather(
                        out_ap=g_t[:, t_lo:t_hi, :],
                        in_ap=xlb[lo:hi, :],
                        idxs_ap=idx_t[:, ic_lo:ic_lo + nidx // 16],
                        num_idxs=nidx, num_idxs_reg=nidx, elem_size=D,
                        single_packet=SINGLE_PACKET,
                        queue_num=b % 4)

                oh_t = ohp.tile([P, max_ntile, P], fp16, tag="onehot")
                i_ap = iota_sb[:]
                iota_b = bass.AP(i_ap.tensor, i_ap.offset,
                                 [i_ap.ap[0], [0, n_t], [1, P]])
                e_ap = elo_t[:]
                elo_b = bass.AP(e_ap.tensor, e_ap.offset,
                                [e_ap.ap[0], [1, n_t], [0, P]])
                nc.vector.tensor_tensor(out=oh_t[:, :n_t, :], in0=iota_b,
                                        in1=elo_b, op=mybir.AluOpType.is_equal)

                h0_t = hcp.tile([P, gcols], fp16, tag="h0")
                c0_t = hcp.tile([P, gcols], fp16, tag="c0")
                nc.sync.dma_start(out=h0_t[:], in_=h0t[:, g * gcols:(g + 1) * gcols])
                nc.sync.dma_start(out=c0_t[:], in_=c0t[:, g * gcols:(g + 1) * gcols])

                i_p = plnp.tile([P, gcols], fp16, tag="i_p")
                f_p = plnp.tile([P, gcols], fp16, tag="f_p")
                g_p = plnp.tile([P, gcols], fp16, tag="g_p")
                o_p = plnp.tile([P, gcols], fp16, tag="o_p")

                # tiles grouped by local chunk
                by_chunk = [[] for _ in range(G)]
                for ti, (b, j) in enumerate(gm["tiles"]):
                    by_chunk[j - g * G].append(ti)

                for s in range(sc_per_g):
                    msg_ps = pmsum.tile([P, SCC * P], f32, tag="msgps")
                    for c in range(SCC):
                        lst = by_chunk[s * SCC + c]
                        for k, ti in enumerate(lst):
                            nc.tensor.matmul(
                                out=msg_ps[:, c * P:(c + 1) * P],
                                lhsT=g_t[:, ti, :],
                                rhs=oh_t[:, ti, :],
                                start=(k == 0),
                                stop=(k == len(lst) - 1),
                            )
                    msg_sb = msgp.tile([P, SCC * P], fp16, tag="msgsb")
                    nc.vector.tensor_copy(out=msg_sb[:], in_=msg_ps[:])

                    sco = s * SCC * P       # col offset of sc in group
                    scs = slice(sco, sco + SCC * P)
                    ga = pgate.tile([P, 2 * SCC * P], f32, tag="gateA")
                    gb = pgate.tile([P, 2 * SCC * P], f32, tag="gateB")
                    for gi, (ps, half) in enumerate(
                            [(ga, 0), (ga, 1), (gb, 0), (gb, 1)]):
                        gs = slice(half * SCC * P, (half + 1) * SCC * P)
                        nc.tensor.matmul(out=ps[:, gs],
                                         lhsT=wih_sb[:, gi * D:(gi + 1) * D],
                                         rhs=msg_sb[:], start=True, stop=False)
                        nc.tensor.matmul(out=ps[:, gs],
                                         lhsT=whh_sb[:, gi * D:(gi + 1) * D],
                                         rhs=h0_t[:, scs], start=False, stop=True)
                    hw = SCC * P
                    nc.scalar.activation(i_p[:, scs], ga[:, 0:hw], Sig,
                                         bias=b_sb[:, 0:1])
                    nc.scalar.activation(f_p[:, scs], ga[:, hw:2 * hw], Sig,
                                         bias=b_sb[:, 1:2])
                    nc.scalar.activation(g_p[:, scs], gb[:, 0:hw], Tnh,
                                         bias=b_sb[:, 2:3])
                    nc.scalar.activation(o_p[:, scs], gb[:, hw:2 * hw], Sig,
                                         bias=b_sb[:, 3:4])

                # group pointwise + outputs
                t1 = tmpp.tile([P, gcols], fp16, tag="t1")
                t2 = tmpp.tile([P, gcols], fp16, tag="t2")
                tnh = tmpp.tile([P, gcols], fp16, tag="tnh")
                ct_o = outp.tile([P, gcols], fp16, tag="ct_o")
                ht_o = outp.tile([P, gcols], fp16, tag="ht_o")
                nc.vector.tensor_mul(out=t1[:], in0=f_p[:], in1=c0_t[:])
                nc.vector.tensor_mul(out=t2[:], in0=i_p[:], in1=g_p[:])
                nc.vector.tensor_add(out=ct_o[:], in0=t1[:], in1=t2[:])
                nc.scalar.activation(tnh[:], ct_o[:], Tnh)
                nc.vector.tensor_mul(out=ht_o[:], in0=o_p[:], in1=tnh[:])

                nc.sync.dma_start(out=ht[:, g * gcols:(g + 1) * gcols], in_=ht_o[:])
                nc.sync.dma_start(out=ct[:, g * gcols:(g + 1) * gcols], in_=ct_o[:])

    nc.compile()
    return nc


def _prep_core_inputs(core, inputs, st, cells):
    h0, c0 = inputs["h0"], inputs["c0"]
    n_chunks = st["n_chunks"]
    budgets = st["budgets"]
    ncols = n_chunks * P
    cpc = CPC
    c_lo = core * cpc

    h0t = np.zeros((P, ncols), dtype=np.float16)
    c0t = np.zeros((P, ncols), dtype=np.float16)
    h0t[:, :cpc] = h0[c_lo:c_lo + cpc].T.astype(np.float16)
    c0t[:, :cpc] = c0[c_lo:c_lo + cpc].T.astype(np.float16)

    geoms, total_icols, total_tiles = _group_geometry(st)
    eidx = np.zeros((P, total_icols), dtype=np.int16)
    eloc_flat = np.full(total_tiles * P, -1.0, dtype=np.float16)

    for g, gm in enumerate(geoms):
        for (b, t_lo, t_hi, ic_lo, nidx, j0, j1) in gm["calls"]:
            flat = np.full(nidx, -1 if NEG_PAD else 0, dtype=np.int16)
            slot0 = (gm["tcol0"] + t_lo) * P
            off = 0
            for j in range(j0, j1):
                bud = int(budgets[j, b])
                if bud == 0:
                    continue
                lits, locs = cells.get((j, b), (None, None))
                if lits is not None:
                    n = len(lits)
                    flat[off:off + n] = lits
                    eloc_flat[slot0 + off:slot0 + off + n] = locs
                off += bud
            blk = flat.reshape(nidx // 16, 16).T
            icol = gm["icol0"] + ic_lo
            for r in range(8):
                eidx[16 * r:16 * (r + 1), icol:icol + nidx // 16] = blk
    eloc = eloc_flat.reshape(total_tiles, P).T.copy()

    return {"xlb": np.ascontiguousarray(inputs["x_l"].astype(np.float16)),
            "h0t": h0t, "c0t": c0t, "eidx": eidx, "eloc": eloc}


def _shared_inputs(inputs):
    W_ih, W_hh = inputs["W_ih"], inputs["W_hh"]
    b2 = (inputs["b_ih"] + inputs["b_hh"]).astype(np.float32)
    wih = np.ascontiguousarray(W_ih.T.astype(np.float16))
    whh = np.ascontiguousarray(W_hh.T.astype(np.float16))
    btab = np.ascontiguousarray(b2.reshape(4, P).T)
    iota = np.broadcast_to(np.arange(P, dtype=np.float16), (P, P))
    return {"wih": wih, "whh": whh, "btab": btab,
            "iota": np.ascontiguousarray(iota)}


def kernel(x_l, h0, c0, W_ih, W_hh, b_ih, b_hh, edge_lit, edge_clause):
    from concourse.bass_utils import run_bass_kernel_spmd

    inputs = dict(x_l=x_l, h0=h0, c0=c0, W_ih=W_ih, W_hh=W_hh, b_ih=b_ih,
                  b_hh=b_hh, edge_lit=edge_lit, edge_clause=edge_clause)

    st, per_core = _compute_structure(np.asarray(edge_lit),
                                      np.asarray(edge_clause))
    key = ("v5", st["n_chunks"], st["n_banks"], st["budgets"].tobytes(),
           MAX_CALL_IDX, SINGLE_PACKET, G, NEG_PAD, DMA_SCRATCH)
    if key not in _cache:
        _cache[key] = _build_program(st)
    nc = _cache[key]

    shared = _shared_inputs(inputs)
    in_maps = []
    for k in range(N_CORES):
        m = _prep_core_inputs(k, inputs, st, per_core[k])
        m.update(shared)
        in_maps.append(m)

    res = run_bass_kernel_spmd(nc, in_maps, core_ids=list(range(N_CORES)),
                               trace=TRACE)
    global LAST_RESULT
    LAST_RESULT = res

    h_new = np.empty((N_CLAUSE, D), dtype=np.float32)
    c_new = np.empty((N_CLAUSE, D), dtype=np.float32)
    for k in range(N_CORES):
        out = res.results[k]
        h_new[k * CPC:(k + 1) * CPC] = out["ht"][:, :CPC].T.astype(np.float32)
        c_new[k * CPC:(k + 1) * CPC] = out["ct"][:, :CPC].T.astype(np.float32)
    return (h_new, c_new)


# revision 12
# speedup vs baseline: 1.4039x; 1.4039x over previous
"""Trainium2 Bass kernel for nn_LitToClauseLayer (gather + segment_sum + LSTM cell).

Reference computation:
    msg   = segment_sum(x_l[edge_lit], edge_clause, num_segments=N_CLAUSE)   # [NC, D]
    gates = msg @ W_ih.T + b_ih + h0 @ W_hh.T + b_hh                         # [NC, 4D]
    i, f, g, o = split(gates); i,f,o = sigmoid; g = tanh
    c_new = f*c0 + i*g ; h_new = o*tanh(c_new)
    returns (h_new, c_new)

Distribution (8 cores, SPMD): clauses + clause-sorted edges sharded across
cores; x_l (fp16) and LSTM weights replicated.  On-device layout is
feature-major ([D=128 partitions, clause]).

v5 structure (vs v4 baseline):
  - Gather calls are packed to up to MAX_CALL_IDX indices each (v4 used
    <=1024 because single_packet=True caps at 64 descs/engine) so the
    ~1us fixed SWDGE ucode overhead per call amortizes.
  - Work is grouped by G=16 clause chunks (2048 clauses): one gather tile,
    one one-hot is_equal, one h0/c0 load, batched pointwise + tanh ops and
    one output store per group; gates stream 512 cols per matmul.
"""

import numpy as np

N_LIT, N_CLAUSE, N_EDGES, D = 100000, 400000, 1200000, 128
N_CORES = 8
CPC = N_CLAUSE // N_CORES  # clauses per core (50000)
P = 128
BANK_ROWS = 25000
G = 16                     # chunks per group
SCC = 4                    # chunks per superchunk (PSUM msg tile)
MAX_CALL_IDX = 512         # pack gather calls up to this many indices
                           # (single_packet caps at 16KB/engine = 1024x256B;
                           # ring is 16KB/queue so ~512 keeps 2 calls in flight)
SINGLE_PACKET = True
NEG_PAD = False            # pad slots use idx=-1 (probe: ucode skip?)
DMA_SCRATCH = 16384        # descriptor ring carveout bytes/partition

_cache = {}

TRACE = False
LAST_RESULT = None


def _compute_structure(edge_lit, edge_clause, n_lit=N_LIT, cpc=CPC,
                       bank_rows=BANK_ROWS, n_cores=N_CORES):
    """Shared (SPMD) program structure + per-core edge placement."""
    n_banks = -(-n_lit // bank_rows)
    n_chunks = -(-cpc // P)
    n_chunks = -(-n_chunks // G) * G
    n_groups = n_chunks // G

    counts = np.zeros((n_cores, n_chunks, n_banks), dtype=np.int64)
    per_core = []
    for k in range(n_cores):
        marks = k * cpc + P * np.arange(n_chunks + 1)
        marks = np.minimum(marks, (k + 1) * cpc)
        bounds = np.searchsorted(edge_clause, marks, side="left")
        cells = {}
        for j in range(n_chunks):
            e0, e1 = bounds[j], bounds[j + 1]
            if e1 > e0:
                lits = edge_lit[e0:e1]
                locs = (edge_clause[e0:e1] - (k * cpc + j * P)).astype(np.int16)
                banks = lits // bank_rows
                order = np.argsort(banks, kind="stable")
                lits, locs, banks = lits[order], locs[order], banks[order]
                for b in range(n_banks):
                    m = banks == b
                    if m.any():
                        cells[(j, b)] = (lits[m] - b * bank_rows, locs[m])
                        counts[k, j, b] = int(m.sum())
        per_core.append(cells)

    maxc = counts.max(axis=0)                       # [n_chunks, n_banks]
    budgets = ((maxc + P - 1) // P) * P             # slots per cell
    for j in range(n_chunks):                       # every chunk >= 1 tile
        if budgets[j].sum() == 0:
            budgets[j, 0] = P

    # Pack gather calls: per (group, bank), consecutive chunk-cells packed
    # into calls of <= MAX_CALL_IDX indices (cell budgets are 128-multiples).
    # calls: list of (group, bank, chunk_lo, chunk_hi, nidx)
    calls = []
    for g in range(n_groups):
        for b in range(n_banks):
            j = g * G
            while j < (g + 1) * G:
                nidx = 0
                j0 = j
                while j < (g + 1) * G and nidx + budgets[j, b] <= MAX_CALL_IDX:
                    nidx += int(budgets[j, b])
                    j += 1
                assert j > j0, f"cell budget {budgets[j, b]} > MAX_CALL_IDX"
                if nidx > 0:
                    calls.append((g, b, j0, j, nidx))

    structure = {
        "n_lit": n_lit, "n_banks": n_banks, "bank_rows": bank_rows,
        "n_chunks": n_chunks, "n_groups": n_groups,
        "budgets": budgets, "calls": calls,
    }
    return structure, per_core


def _group_geometry(st):
    """Per-group tile lists and index-column layout (compile-time, SPMD).

    Tile order within a group: bank-major, then chunk, matching the call
    packing so each call writes a contiguous tile range of the group tile.
    Returns per-group dicts.
    """
    budgets = st["budgets"]
    n_banks, n_groups = st["n_banks"], st["n_groups"]
    geoms = []
    icol = 0        # global eidx column (16 idx per column)
    tcol = 0        # global eloc tile index
    for g in range(n_groups):
        tiles = []          # (bank, chunk_global) per 128-slot tile
        gcalls = []         # (bank, tile_lo, tile_hi, icol_lo, nidx, chunk_lo, chunk_hi)
        icol0, tcol0 = icol, tcol
        for (cg, b, j0, j1, nidx) in st["calls"]:
            if cg != g:
                continue
            tile_lo = len(tiles)
            for j in range(j0, j1):
                for _ in range(int(budgets[j, b]) // P):
                    tiles.append((b, j))
            gcalls.append((b, tile_lo, len(tiles), icol - icol0, nidx, j0, j1))
            icol += nidx // 16
        tcol += len(tiles)
        geoms.append({
            "tiles": tiles, "calls": gcalls,
            "icol0": icol0, "icols": icol - icol0,
            "tcol0": tcol0, "ntile": len(tiles),
        })
    return geoms, icol, tcol


def _build_program(st):
    import concourse.bacc as bacc
    import concourse.bass as bass
    import concourse.mybir as mybir
    import concourse.tile as tile

    dt = mybir.dt
    n_chunks, n_groups = st["n_chunks"], st["n_groups"]
    n_banks, bank_rows = st["n_banks"], st["bank_rows"]
    budgets = st["budgets"]
    n_lit = st["n_lit"]
    ncols = n_chunks * P
    gcols = G * P
    sc_per_g = G // SCC

    geoms, total_icols, total_tiles = _group_geometry(st)
    max_icols = max(gm["icols"] for gm in geoms)
    max_ntile = max(gm["ntile"] for gm in geoms)

    nc = bacc.Bacc(None, target_bir_lowering=False, num_swdge_queues=4,
                   dynamic_dma_scratch_size=DMA_SCRATCH)

    xlb = nc.dram_tensor("xlb", [n_lit, D], dt.float16, kind="ExternalInput")
    h0t = nc.dram_tensor("h0t", [P, ncols], dt.float16, kind="ExternalInput")
    c0t = nc.dram_tensor("c0t", [P, ncols], dt.float16, kind="ExternalInput")
    eidx = nc.dram_tensor("eidx", [P, total_icols], dt.int16, kind="ExternalInput")
    eloc = nc.dram_tensor("eloc", [P, total_tiles], dt.float16, kind="ExternalInput")
    wih = nc.dram_tensor("wih", [P, 4 * D], dt.float16, kind="ExternalInput")
    whh = nc.dram_tensor("whh", [P, 4 * D], dt.float16, kind="ExternalInput")
    btab = nc.dram_tensor("btab", [P, 4], dt.float32, kind="ExternalInput")
    iota = nc.dram_tensor("iota", [P, P], dt.float16, kind="ExternalInput")
    ht = nc.dram_tensor("ht", [P, ncols], dt.float16, kind="ExternalOutput")
    ct = nc.dram_tensor("ct", [P, ncols], dt.float16, kind="ExternalOutput")

    f32, fp16, i16 = dt.float32, dt.float16, dt.int16
    Sig = mybir.ActivationFunctionType.Sigmoid
    Tnh = mybir.ActivationFunctionType.Tanh

    with tile.TileContext(nc) as tc:
        with (
            tc.tile_pool(name="const", bufs=1) as cpool,
            tc.tile_pool(name="tabs", bufs=2) as tabs,
            tc.tile_pool(name="gat", bufs=2) as gat,
            tc.tile_pool(name="oh", bufs=2) as ohp,
            tc.tile_pool(name="hc", bufs=2) as hcp,
            tc.tile_pool(name="msg", bufs=2) as msgp,
            tc.tile_pool(name="pln", bufs=2) as plnp,
            tc.tile_pool(name="tmp", bufs=2) as tmpp,
            tc.tile_pool(name="outs", bufs=2) as outp,
            tc.tile_pool(name="pm", bufs=2, space="PSUM") as pmsum,
            tc.tile_pool(name="pg", bufs=1, space="PSUM") as pgate,
        ):
            wih_sb = cpool.tile([P, 4 * D], fp16, tag="wih")
            whh_sb = cpool.tile([P, 4 * D], fp16, tag="whh")
            b_sb = cpool.tile([P, 4], f32, tag="btab")
            iota_sb = cpool.tile([P, P], fp16, tag="iota")
            nc.sync.dma_start(out=wih_sb[:], in_=wih[:, :])
            nc.sync.dma_start(out=whh_sb[:], in_=whh[:, :])
            nc.sync.dma_start(out=b_sb[:], in_=btab[:, :])
            nc.sync.dma_start(out=iota_sb[:], in_=iota[:, :])

            for g in range(n_groups):
                gm = geoms[g]
                n_t = gm["ntile"]
                icols = gm["icols"]

                idx_t = tabs.tile([P, max(max_icols, 16)], i16, tag="idx")
                nc.sync.dma_start(out=idx_t[:, :icols],
                                  in_=eidx[:, gm["icol0"]:gm["icol0"] + icols])
                elo_t = tabs.tile([P, max_ntile], fp16, tag="elo")
                nc.sync.dma_start(out=elo_t[:, :n_t],
                                  in_=eloc[:, gm["tcol0"]:gm["tcol0"] + n_t])

                g_t = gat.tile([P, max_ntile, D], fp16, tag="gt")
                # interleave banks in issue order and rotate queues:
                # consecutive same-queue calls stall on ring reclaim
                by_bank = {}
                for i, c in enumerate(gm["calls"]):
                    by_bank.setdefault(c[0], []).append(i)
                order = []
                while any(by_bank.values()):
                    for b in sorted(by_bank):
                        if by_bank[b]:
                            order.append(by_bank[b].pop(0))
                for qi, ci in enumerate(order):
                    (b, t_lo, t_hi, ic_lo, nidx, j0, j1) = gm["calls"][ci]
                    lo = b * bank_rows
                    hi = min(lo + bank_rows, n_lit)
                    nc.gpsimd.dma_gather(
                        out_ap=g_t[:, t_lo:t_hi, :],
                        in_ap=xlb[lo:hi, :],
                        idxs_ap=idx_t[:, ic_lo:ic_lo + nidx // 16],
                        num_idxs=nidx, num_idxs_reg=nidx, elem_size=D,
                        single_packet=SINGLE_PACKET,
                        queue_num=qi % 4)

                oh_t = ohp.tile([P, max_ntile, P], fp16, tag="onehot")
                i_ap = iota_sb[:]
                iota_b = bass.AP(i_ap.tensor, i_ap.offset,
                                 [i_ap.ap[0], [0, n_t], [1, P]])
                e_ap = elo_t[:]
                elo_b = bass.AP(e_ap.tensor, e_ap.offset,
                                [e_ap.ap[0], [1, n_t], [0, P]])
                nc.vector.tensor_tensor(out=oh_t[:, :n_t, :], in0=iota_b,
                                        in1=elo_b, op=mybir.AluOpType.is_equal)

                h0_t = hcp.tile([P, gcols], fp16, tag="h0")
                c0_t = hcp.tile([P, gcols], fp16, tag="c0")
                nc.sync.dma_start(out=h0_t[:], in_=h0t[:, g * gcols:(g + 1) * gcols])
                nc.sync.dma_start(out=c0_t[:], in_=c0t[:, g * gcols:(g + 1) * gcols])

                i_p = plnp.tile([P, gcols], fp16, tag="i_p")
                f_p = plnp.tile([P, gcols], fp16, tag="f_p")
                g_p = plnp.tile([P, gcols], fp16, tag="g_p")
                o_p = plnp.tile([P, gcols], fp16, tag="o_p")

                # tiles grouped by local chunk
                by_chunk = [[] for _ in range(G)]
                for ti, (b, j) in enumerate(gm["tiles"]):
                    by_chunk[j - g * G].append(ti)

                for s in range(sc_per_g):
                    msg_ps = pmsum.tile([P, SCC * P], f32, tag="msgps")
                    for c in range(SCC):
                        lst = by_chunk[s * SCC + c]
                        for k, ti in enumerate(lst):
                            nc.tensor.matmul(
                                out=msg_ps[:, c * P:(c + 1) * P],
                                lhsT=g_t[:, ti, :],
                                rhs=oh_t[:, ti, :],
                                start=(k == 0),
                                stop=(k == len(lst) - 1),
                            )
                    msg_sb = msgp.tile([P, SCC * P], fp16, tag="msgsb")
                    nc.vector.tensor_copy(out=msg_sb[:], in_=msg_ps[:])

                    sco = s * SCC * P       # col offset of sc in group
                    scs = slice(sco, sco + SCC * P)
                    ga = pgate.tile([P, 2 * SCC * P], f32, tag="gateA")
                    gb = pgate.tile([P, 2 * SCC * P], f32, tag="gateB")
                    for gi, (ps, half) in enumerate(
                            [(ga, 0), (ga, 1), (gb, 0), (gb, 1)]):
                        gs = slice(half * SCC * P, (half + 1) * SCC * P)
                        nc.tensor.matmul(out=ps[:, gs],
                                         lhsT=wih_sb[:, gi * D:(gi + 1) * D],
                                         rhs=msg_sb[:], start=True, stop=False)
                        nc.tensor.matmul(out=ps[:, gs],
                                         lhsT=whh_sb[:, gi * D:(gi + 1) * D],
                                         rhs=h0_t[:, scs], start=False, stop=True)
                    hw = SCC * P
                    nc.scalar.activation(i_p[:, scs], ga[:, 0:hw], Sig,
                                         bias=b_sb[:, 0:1])
                    nc.scalar.activation(f_p[:, scs], ga[:, hw:2 * hw], Sig,
                                         bias=b_sb[:, 1:2])
                    nc.scalar.activation(g_p[:, scs], gb[:, 0:hw], Tnh,
                                         bias=b_sb[:, 2:3])
                    nc.scalar.activation(o_p[:, scs], gb[:, hw:2 * hw], Sig,
                                         bias=b_sb[:, 3:4])

                # group pointwise + outputs
                t1 = tmpp.tile([P, gcols], fp16, tag="t1")
                t2 = tmpp.tile([P, gcols], fp16, tag="t2")
                tnh = tmpp.tile([P, gcols], fp16, tag="tnh")
                ct_o = outp.tile([P, gcols], fp16, tag="ct_o")
                ht_o = outp.tile([P, gcols], fp16, tag="ht_o")
                nc.vector.tensor_mul(out=t1[:], in0=f_p[:], in1=c0_t[:])
                nc.vector.tensor_mul(out=t2[:], in0=i_p[:], in1=g_p[:])
                nc.vector.tensor_add(out=ct_o[:], in0=t1[:], in1=t2[:])
                nc.scalar.activation(tnh[:], ct_o[:], Tnh)
                nc.vector.tensor_mul(out=ht_o[:], in0=o_p[:], in1=tnh[:])

                nc.sync.dma_start(out=ht[:, g * gcols:(g + 1) * gcols], in_=ht_o[:])
                nc.sync.dma_start(out=ct[:, g * gcols:(g + 1) * gcols], in_=ct_o[:])

    nc.compile()
    return nc


def _prep_core_inputs(core, inputs, st, cells):
    h0, c0 = inputs["h0"], inputs["c0"]
    n_chunks = st["n_chunks"]
    budgets = st["budgets"]
    ncols = n_chunks * P
    cpc = CPC
    c_lo = core * cpc

    h0t = np.zeros((P, ncols), dtype=np.float16)
    c0t = np.zeros((P, ncols), dtype=np.float16)
    h0t[:, :cpc] = h0[c_lo:c_lo + cpc].T.astype(np.float16)
    c0t[:, :cpc] = c0[c_lo:c_lo + cpc].T.astype(np.float16)

    geoms, total_icols, total_tiles = _group_geometry(st)
    eidx = np.zeros((P, total_icols), dtype=np.int16)
    eloc_flat = np.full(total_tiles * P, -1.0, dtype=np.float16)

    for g, gm in enumerate(geoms):
        for (b, t_lo, t_hi, ic_lo, nidx, j0, j1) in gm["calls"]:
            flat = np.full(nidx, -1 if NEG_PAD else 0, dtype=np.int16)
            slot0 = (gm["tcol0"] + t_lo) * P
            off = 0
            for j in range(j0, j1):
                bud = int(budgets[j, b])
                if bud == 0:
                    continue
                lits, locs = cells.get((j, b), (None, None))
                if lits is not None:
                    n = len(lits)
                    flat[off:off + n] = lits
                    eloc_flat[slot0 + off:slot0 + off + n] = locs
                off += bud
            blk = flat.reshape(nidx // 16, 16).T
            icol = gm["icol0"] + ic_lo
            for r in range(8):
                eidx[16 * r:16 * (r + 1), icol:icol + nidx // 16] = blk
    eloc = eloc_flat.reshape(total_tiles, P).T.copy()

    return {"xlb": np.ascontiguousarray(inputs["x_l"].astype(np.float16)),
            "h0t": h0t, "c0t": c0t, "eidx": eidx, "eloc": eloc}


def _shared_inputs(inputs):
    W_ih, W_hh = inputs["W_ih"], inputs["W_hh"]
    b2 = (inputs["b_ih"] + inputs["b_hh"]).astype(np.float32)
    wih = np.ascontiguousarray(W_ih.T.astype(np.float16))
    whh = np.ascontiguousarray(W_hh.T.astype(np.float16))
    btab = np.ascontiguousarray(b2.reshape(4, P).T)
    iota = np.broadcast_to(np.arange(P, dtype=np.float16), (P, P))
    return {"wih": wih, "whh": whh, "btab": btab,
            "iota": np.ascontiguousarray(iota)}


def kernel(x_l, h0, c0, W_ih, W_hh, b_ih, b_hh, edge_lit, edge_clause):
    from concourse.bass_utils import run_bass_kernel_spmd

    inputs = dict(x_l=x_l, h0=h0, c0=c0, W_ih=W_ih, W_hh=W_hh, b_ih=b_ih,
                  b_hh=b_hh, edge_lit=edge_lit, edge_clause=edge_clause)

    st, per_core = _compute_structure(np.asarray(edge_lit),
                                      np.asarray(edge_clause))
    key = ("v5", st["n_chunks"], st["n_banks"], st["budgets"].tobytes(),
           MAX_CALL_IDX, SINGLE_PACKET, G, NEG_PAD, DMA_SCRATCH)
    if key not in _cache:
        _cache[key] = _build_program(st)
    nc = _cache[key]

    shared = _shared_inputs(inputs)
    in_maps = []
    for k in range(N_CORES):
        m = _prep_core_inputs(k, inputs, st, per_core[k])
        m.update(shared)
        in_maps.append(m)

    res = run_bass_kernel_spmd(nc, in_maps, core_ids=list(range(N_CORES)),
                               trace=TRACE)
    global LAST_RESULT
    LAST_RESULT = res

    h_new = np.empty((N_CLAUSE, D), dtype=np.float32)
    c_new = np.empty((N_CLAUSE, D), dtype=np.float32)
    for k in range(N_CORES):
        out = res.results[k]
        h_new[k * CPC:(k + 1) * CPC] = out["ht"][:, :CPC].T.astype(np.float32)
        c_new[k * CPC:(k + 1) * CPC] = out["ct"][:, :CPC].T.astype(np.float32)
    return (h_new, c_new)
